# revision 29
# baseline (speedup 1.0000x reference)
"""Trainium2 Bass kernel for nn_BinaryLinear (binary-weight linear + BatchNorm + sign).

Computation (reference):
    bw    = sign(W)                     # [O, I], entries in {-1, 0, +1}
    alpha = mean(|W|, axis=1)           # [O]
    y     = x @ (bw * alpha).T          # [B, O]
    out   = sign((y - mu_b) / sqrt(var_b + eps) * gamma + beta)   # batch stats

Strategy (8 NeuronCores, column-sharded):
  * Each core owns O/8 = 512 output columns; BN batch stats are then fully
    local to a core (full batch for its columns) -> no collectives.
  * alpha is factored out of the matmul: s = x @ bw.T is computed on the PE
    in a SINGLE float32r pass. HW-probed: f32r matmul streams at ~227 ns per
    128x512 MM and rounds operands RNE to 11 mantissa bits (FP22). Measured
    end-to-end: 1078 sign flips, rel_err 0.0113 -- comfortably under the
    2e-2 gate at HALF the PE work of a bf16 hi/lo two-pass scheme.
  * x is fed as chunk-PAIR tiles [128, 2*CH]: each i-tile's weights feed 2
    back-to-back matmuls (all 8 PSUM banks live), so the 4-byte f32r
    LDWEIGHTS (~190 ns, no FWL) hides under ~2 matmuls.
  * Weights are stored bf16 in SBUF (32 KB/partition) and upconverted
    per-cpair to f32 scratch tiles by DVE (matmul reads them via a f32r
    bitcast). This frees enough SBUF to keep ALL of y.T (16 MB) resident --
    no DRAM spill/readback -- and halves weight DMA.
  * Layout is transposed on host: s.T[o, b] so o sits on SBUF partitions.
    BN stats are per-partition reductions along the free dim (BN_STATS /
    BN_AGGR); the final affine+sign splits between ScalarE and DVE.
  * PSUM drain at each cpair boundary is split: BN_STATS on DVE, the y
    evacuation copies on ScalarE, so banks free in ~5us not ~19us.
"""

import os
from contextlib import ExitStack

import ml_dtypes
import numpy as np

import concourse.bacc as bacc
import concourse.bass as bass
import concourse.mybir as mybir
import concourse.tile as tile
from concourse.bass_utils import run_bass_kernel_spmd

BF16 = ml_dtypes.bfloat16
BN_EPS = 1e-5

N_CORES = 8
B_FULL, IN_F, OUT_F = 8192, 4096, 4096

LAST_RESULTS = None  # BassKernelResults of the most recent device run

CV_BUFS = 5  # f32 weight-conversion scratch tiles (DVE prefetch depth)


def build_nc(B, I, OSH, CH=512, xbufs=4, simple_tail=False):
    """Build + compile the per-core Bass program.

    B: batch (free dim of s.T), I: contraction, OSH: output columns per core,
    CH: batch chunk (<=512, PSUM bank / bn_stats limit). simple_tail may only
    be set when gamma > 0 and beta == 0 (sign(BN(y)) == sign(s - mean_s)).
    """
    NOT = OSH // 128          # o-tiles (PSUM partition groups)
    NT = I // 128             # i-tiles (contraction)
    NCH = B // CH             # batch chunks
    NCP = NCH // 2            # chunk pairs
    f32 = mybir.dt.float32
    f32r = mybir.dt.float32r
    bf16 = mybir.dt.bfloat16

    nc = bacc.Bacc("TRN2", target_bir_lowering=False, debug=False)
    xtp_d = nc.dram_tensor(
        "xtp", [NT, NCP, 128, 2 * CH], f32r, kind="ExternalInput"
    )
    bwt_d = nc.dram_tensor("bwt", [NT, 128, OSH], bf16, kind="ExternalInput")
    coef_d = nc.dram_tensor("coef", [128, 4 * NOT], f32, kind="ExternalInput")
    out_d = nc.dram_tensor("out", [OSH, B], bf16, kind="ExternalOutput")
    SGW = min(B, 2048)        # sign-pass slab width
    NSG = B // SGW

    with tile.TileContext(nc) as tc, ExitStack() as ctx:
        bw_pool = ctx.enter_context(tc.tile_pool(name="bw", bufs=NT))
        cv_pool = ctx.enter_context(tc.tile_pool(name="cv", bufs=CV_BUFS))
        x_pool = ctx.enter_context(tc.tile_pool(name="x", bufs=xbufs))
        y_pool = ctx.enter_context(tc.tile_pool(name="y", bufs=1))
        ps_pool = ctx.enter_context(
            tc.tile_pool(name="ps", bufs=8, space=bass.MemorySpace.PSUM)
        )
        st_pool = ctx.enter_context(tc.tile_pool(name="st", bufs=1))
        sg_pool = ctx.enter_context(tc.tile_pool(name="sg", bufs=4))
        sm_pool = ctx.enter_context(tc.tile_pool(name="sm", bufs=NOT))

        # PE warm-up: the HAM clock gate holds the PE at 1.2 GHz until it has
        # been busy ~3.4us. Burn dummy matmuls during the initial DMA wait so
        # the real matmul stream starts at 2.4 GHz.
        wl = sm_pool.tile([128, 64], bf16)
        wr = sm_pool.tile([128, 64], bf16)
        nc.vector.memset(wl[:], 0.0)
        nc.vector.memset(wr[:], 0.0)
        # 24 x ~110ns cold covers the DMA/conversion latency of the first
        # real tiles without queueing so much warmup that it delays them
        wp = ps_pool.tile([128, CH], f32, name="wups", tag="ps")
        for _ in range(24):
            nc.tensor.matmul(wp[0:64, 0:64], wl[:], wr[:], start=True, stop=True)

        bw_tiles = [None] * NT
        # All weight tiles ride the ACT HWDGE ring: ACT is idle at startup,
        # while Sync and GpSimd carry the x stream from t=0 (queuing bw
        # behind x there starved the first chunk-pair's conversions).
        for t in range(NT):
            bt = bw_pool.tile([128, OSH], bf16, name=f"bt{t}", tag="bt")
            nc.scalar.dma_start(bt[:], bwt_d.ap()[t])
            bw_tiles[t] = bt

        ct = sm_pool.tile([128, 4 * NOT], f32)
        nc.gpsimd.dma_start(ct[:], coef_d.ap())

        yt = [y_pool.tile([128, B], f32, name=f"yt{i}") for i in range(NOT)]
        stats = [st_pool.tile([128, 6 * NCH], f32, name=f"stats{i}") for i in range(NOT)]

        # Per-o-tile BN coefficients: with s-stats (mean_s, var_s) and host
        # precomputed p1=alpha^2, p2=alpha*gamma, p4=beta:
        #   inv = 1/sqrt(p1*var_s + eps);  A = p2*inv;  B = p4 - mean_s*A
        A_t, B_t, mv_t = [None] * NOT, [None] * NOT, [None] * NOT
        eps_t = sm_pool.tile([128, 1], f32)
        nc.vector.memset(eps_t[:], BN_EPS)

        def coef_chain(ot):
            mv = sm_pool.tile([128, 2], f32, name=f"mv{ot}", tag="mv")
            nc.vector.bn_aggr(mv[:], stats[ot][:])
            p1 = ct[:, ot : ot + 1]
            p2 = ct[:, NOT + ot : NOT + ot + 1]
            p4 = ct[:, 3 * NOT + ot : 3 * NOT + ot + 1]
            v = sm_pool.tile([128, 1], f32, name=f"v{ot}", tag="v")
            nc.vector.tensor_mul(v[:], mv[:, 1:2], p1)
            sd = sm_pool.tile([128, 1], f32, name=f"sd{ot}", tag="sd")
            nc.scalar.activation(
                sd[:], v[:], mybir.ActivationFunctionType.Sqrt, bias=eps_t[:]
            )
            inv = sm_pool.tile([128, 1], f32, name=f"inv{ot}", tag="inv")
            nc.vector.reciprocal(inv[:], sd[:])
            Ac = sm_pool.tile([128, 1], f32, name=f"Ac{ot}", tag="Ac")
            nc.vector.tensor_mul(Ac[:], p2, inv[:])
            mB = sm_pool.tile([128, 1], f32, name=f"mB{ot}", tag="mB")
            nc.vector.tensor_mul(mB[:], mv[:, 0:1], Ac[:])
            Bc = sm_pool.tile([128, 1], f32, name=f"Bc{ot}", tag="Bc")
            nc.vector.tensor_sub(Bc[:], p4, mB[:])
            A_t[ot], B_t[ot], mv_t[ot] = Ac, Bc, mv

        # Streaming weight conversion: cv[t] is a f32 copy of bw tile t,
        # refreshed per cpair; matmul reads it through a f32r bitcast.
        cvs = {}

        def conv(cp, t):
            # DVE upconvert bf16 -> f32r (the f32r output dtype makes DVE
            # round to FP22, which the BIR verifier requires for f32r
            # matmul inputs; +-1 weights are exact either way)
            cv = cv_pool.tile([128, OSH], f32r, name=f"cv{cp}_{t}", tag="cv")
            nc.vector.tensor_copy(cv[:], bw_tiles[t][:])
            cvs[t] = cv

        def drain_copies(cp):
            """Evacuate cpair cp's 8 PSUM banks (the copies are the only
            bank readers). Free banks in POOL-ALLOCATION order (ot-major)
            split across ACT/DVE so the next cpair's matmuls reacquire them
            with minimal stall (~0.6us per engine step, two engines)."""
            ps = ps_tiles[cp]
            for ot in range(NOT):
                for j in range(2):
                    c = 2 * cp + j
                    dst = yt[ot][:, c * CH : (c + 1) * CH]
                    if (2 * ot + j) % 2 == 0:
                        nc.scalar.copy(dst, ps[ot][j][:])
                    else:
                        nc.vector.tensor_copy(dst, ps[ot][j][:])

        def drain_stats(cp):
            """BN stats off the bank-free critical path: read the y copies.
            Emitted AFTER the next cpair's conversions so a waiting stats op
            never head-of-line blocks the DVE queue's CAST stream."""
            for ot in range(NOT):
                for j in range(2):
                    c = 2 * cp + j
                    nc.vector.bn_stats(
                        stats[ot][:, c * 6 : (c + 1) * 6],
                        yt[ot][:, c * CH : (c + 1) * CH],
                    )
                    if c == NCH - 1:
                        coef_chain(ot)

        ps_tiles = {}
        for cp in range(NCP):
            if cp == 1:
                # Preload the tail ACT LUTs (Sqrt, Sign) once the stream is
                # rolling: no ACT_TABLE_LOAD on the critical tail.
                wt = sm_pool.tile([128, 1], f32)
                nc.vector.memset(wt[:], 1.0)
                wt2 = sm_pool.tile([128, 1], f32)
                nc.scalar.activation(
                    wt2[:], wt[:], mybir.ActivationFunctionType.Sqrt
                )
                nc.scalar.activation(
                    wt2[:], wt[:], mybir.ActivationFunctionType.Sign
                )
            # convert the first tiles of THIS cpair before emitting the
            # previous cpair's drain, so DVE has them ready when the PE
            # restarts
            for t in range(min(CV_BUFS - 2, NT)):
                conv(cp, t)
            if cp > 0:
                drain_copies(cp - 1)
            # 8 live accumulators: (o-tile, chunk-of-pair) -> all 8 PSUM banks
            ps = [
                [
                    ps_pool.tile([128, CH], f32, name=f"ps{cp}_{i}_{j}", tag="ps")
                    for j in range(2)
                ]
                for i in range(NOT)
            ]
            ps_tiles[cp] = ps
            # x DMAs issued 2 tiles ahead of consumption so each tile's data
            # (and the pulled-ahead LDWEIGHTS) is resident before its first
            # matmul
            xts = {}

            def issue_x(t, cp=cp, xts=xts):
                xt = x_pool.tile([128, 2 * CH], f32r)
                # SP HWDGE + GpSimd SWDGE rings carry x; ACT is reserved for
                # the evacuation copies. Each tile's two 256KB chunk-halves
                # go to BOTH rings at once: the j=0 half (consumed first)
                # lands ~0.6us earlier and per-DMA latency halves.
                e0, e1 = (nc.sync, nc.gpsimd) if t % 2 == 0 else (nc.gpsimd, nc.sync)
                src = xtp_d.ap()[t, cp]
                e0.dma_start(xt[:, 0:CH], src[:, 0:CH])
                e1.dma_start(xt[:, CH : 2 * CH], src[:, CH : 2 * CH])
                xts[t] = xt

            # depth-3 prefetch: each 512KB x DMA gets ~5.5us of issue-to-need
            # slack (depth-2's 3.6us was right at the issue+transfer+receipt
            # latency -> periodic 1-4us PE stalls on the x semaphore)
            issue_x(0)
            issue_x(1)
            issue_x(2)
            for t in range(NT):
                if t + 3 < NT:
                    issue_x(t + 3)
                if t + CV_BUFS - 2 < NT:
                    conv(cp, t + CV_BUFS - 2)
                xt = xts.pop(t)
                cv = cvs.pop(t)
                for ot in range(NOT):
                    lhsT = cv[:, ot * 128 : (ot + 1) * 128]
                    for j in range(2):
                        nc.tensor.matmul(
                            ps[ot][j][:], lhsT, xt[:, j * CH : (j + 1) * CH],
                            start=(t == 0), stop=(t == NT - 1),
                        )
            if cp > 0:
                drain_stats(cp - 1)
        drain_copies(NCP - 1)
        drain_stats(NCP - 1)

        # Final affine+sign, split between ScalarE (one ACTIVATE(Sign) per
        # slab) and DVE so the serial tail drains on two engines at once.
        for k in range(NOT * NSG):
            h, ot = divmod(k, NOT)
            lo = h * SGW
            ysl = yt[ot][:, lo : lo + SGW]
            sg = sg_pool.tile([128, SGW], bf16, name=f"sg{k}", tag="sg")
            # DVE slab (~2.0us) vs ACT slab (~2.1us): 6 of 16 on DVE evens
            # the two engines' tail drain time.
            if k % 8 in (1, 4, 6) and simple_tail:
                # sg = (s >= mean) ; sg = 2*sg - 1
                nc.vector.tensor_scalar(
                    sg[:], ysl, mv_t[ot][:, 0:1], None,
                    mybir.AluOpType.is_ge,
                )
                nc.vector.tensor_scalar(
                    sg[:], sg[:], 2.0, 1.0,
                    mybir.AluOpType.mult, mybir.AluOpType.subtract,
                )
            else:
                nc.scalar.activation(
                    sg[:], ysl,
                    mybir.ActivationFunctionType.Sign,
                    bias=B_t[ot][:],
                    scale=A_t[ot][:],
                )
            # alternate output rings so out-DMAs don't serialize behind one
            # HWDGE queue (frees sg bufs sooner for the shallow pool)
            out_eng = nc.sync if k % 2 == 0 else nc.gpsimd
            out_eng.dma_start(
                out_d.ap()[ot * 128 : (ot + 1) * 128, lo : lo + SGW], sg[:]
            )

    nc.compile()
    return nc


def prep_inputs(x, w, gamma, beta, n_cores=N_CORES, CH=512):
    """Host-side prep: transpose/pair-pack x (f32), bw/coef shards per core."""
    B, I = x.shape
    O = w.shape[0]
    OSH = O // n_cores
    NT = I // 128
    NCH = B // CH

    xt = np.ascontiguousarray(x.T)                  # [I, B] f32
    # chunk-PAIR tiles: [NT, NCH//2, 128, 2CH] (b-contiguous 1024-wide)
    xtp = np.ascontiguousarray(
        xt.reshape(NT, 128, NCH // 2, 2 * CH).transpose(0, 2, 1, 3)
    )

    bw = np.sign(w).astype(np.float32)
    alpha = np.abs(w).mean(axis=1)                 # [O] f32
    p1 = alpha * alpha
    p2 = alpha * gamma
    p3 = alpha * alpha * gamma
    p4 = beta.astype(np.float32)

    in_maps = []
    for k in range(n_cores):
        sl = slice(k * OSH, (k + 1) * OSH)
        bwt = np.ascontiguousarray(bw[sl].T).reshape(NT, 128, OSH).astype(BF16)
        NOT = OSH // 128

        def per_tile(vec):
            return np.ascontiguousarray(vec[sl].reshape(NOT, 128).T)  # [128, NOT]

        coef = np.concatenate(
            [per_tile(p1), per_tile(p2), per_tile(p3), per_tile(p4)], axis=1
        ).astype(np.float32)
        in_maps.append({"xtp": xtp, "bwt": bwt, "coef": coef})
    return in_maps


_NC_CACHE = {}


def kernel(x, real_weight, gamma, beta):
    global LAST_RESULTS
    x = np.asarray(x, dtype=np.float32)
    w = np.asarray(real_weight, dtype=np.float32)
    gamma = np.asarray(gamma, dtype=np.float32)
    beta = np.asarray(beta, dtype=np.float32)
    B, I = x.shape
    O = w.shape[0]
    OSH = O // N_CORES
    CH = 512

    simple_tail = bool((gamma > 0).all() and (beta == 0).all())
    key = (B, I, OSH, CH, simple_tail)
    if key not in _NC_CACHE:
        _NC_CACHE[key] = build_nc(B, I, OSH, CH, simple_tail=simple_tail)
    nc = _NC_CACHE[key]

    in_maps = prep_inputs(x, w, gamma, beta, N_CORES, CH)
    trace = bool(int(os.environ.get("KERNEL_TRACE", "0")))
    res = run_bass_kernel_spmd(
        nc, in_maps, core_ids=list(range(N_CORES)), trace=trace
    )
    LAST_RESULTS = res

    out = np.empty((B, O), dtype=np.float32)
    for k in range(N_CORES):
        o = res.results[k]["out"]                  # [OSH, B] bf16
        out[:, k * OSH : (k + 1) * OSH] = o.T.astype(np.float32)
    return out


# revision 34
# speedup vs baseline: 1.0853x; 1.0853x over previous
"""Trainium2 Bass kernel for nn_BinaryLinear (binary-weight linear + BatchNorm + sign).

Computation (reference):
    bw    = sign(W)                     # [O, I], entries in {-1, 0, +1}
    alpha = mean(|W|, axis=1)           # [O]
    y     = x @ (bw * alpha).T          # [B, O]
    out   = sign((y - mu_b) / sqrt(var_b + eps) * gamma + beta)   # batch stats

Strategy (8 NeuronCores, column-sharded):
  * Each core owns O/8 = 512 output columns; BN batch stats are then fully
    local to a core (full batch for its columns) -> no collectives.
  * alpha is factored out of the matmul: s = x @ bw.T is computed on the PE
    in a SINGLE float32r pass. HW-probed: f32r matmul streams at ~227 ns per
    128x512 MM and rounds operands RNE to 11 mantissa bits (FP22). Measured
    end-to-end: 1078 sign flips, rel_err 0.0113 -- comfortably under the
    2e-2 gate at HALF the PE work of a bf16 hi/lo two-pass scheme.
  * x is fed as chunk-PAIR tiles [128, 2*CH]: each i-tile's weights feed 2
    back-to-back matmuls (all 8 PSUM banks live), so the 4-byte f32r
    LDWEIGHTS (~190 ns, no FWL) hides under ~2 matmuls.
  * Weights are stored bf16 in SBUF (32 KB/partition) and upconverted
    per-cpair to f32 scratch tiles by DVE (matmul reads them via a f32r
    bitcast). This frees enough SBUF to keep ALL of y.T (16 MB) resident --
    no DRAM spill/readback -- and halves weight DMA.
  * Layout is transposed on host: s.T[o, b] so o sits on SBUF partitions.
    BN stats are per-partition reductions along the free dim (BN_STATS /
    BN_AGGR); the final affine+sign splits between ScalarE and DVE.
  * PSUM drain at each cpair boundary is split: BN_STATS on DVE, the y
    evacuation copies on ScalarE, so banks free in ~5us not ~19us.
"""

import os
from contextlib import ExitStack

import ml_dtypes
import numpy as np

import concourse.bacc as bacc
import concourse.bass as bass
import concourse.mybir as mybir
import concourse.tile as tile
from concourse.bass_utils import run_bass_kernel_spmd

BF16 = ml_dtypes.bfloat16
BN_EPS = 1e-5

N_CORES = 8
B_FULL, IN_F, OUT_F = 8192, 4096, 4096

LAST_RESULTS = None  # BassKernelResults of the most recent device run

CV_BUFS = 4  # f32 weight-conversion scratch tiles (DVE prefetch depth)


def build_nc(B, I, OSH, CH=512, xbufs=5, simple_tail=False):
    """Build + compile the per-core Bass program.

    B: batch (free dim of s.T), I: contraction, OSH: output columns per core,
    CH: batch chunk (<=512, PSUM bank / bn_stats limit). simple_tail may only
    be set when gamma > 0 and beta == 0 (sign(BN(y)) == sign(s - mean_s)).
    """
    NOT = OSH // 128          # o-tiles (PSUM partition groups)
    NT = I // 128             # i-tiles (contraction)
    NCH = B // CH             # batch chunks
    NCP = NCH // 2            # chunk pairs
    f32 = mybir.dt.float32
    f32r = mybir.dt.float32r
    bf16 = mybir.dt.bfloat16

    nc = bacc.Bacc("TRN2", target_bir_lowering=False, debug=False)
    xtp_d = nc.dram_tensor(
        "xtp", [NT, NCP, 128, 2 * CH], f32r, kind="ExternalInput"
    )
    bwt_d = nc.dram_tensor("bwt", [NT, 128, OSH], bf16, kind="ExternalInput")
    coef_d = nc.dram_tensor("coef", [128, 4 * NOT], f32, kind="ExternalInput")
    out_d = nc.dram_tensor("out", [OSH, B], bf16, kind="ExternalOutput")
    SGW = min(B, 2048)        # sign-pass slab width
    NSG = B // SGW

    with tile.TileContext(nc) as tc, ExitStack() as ctx:
        bw_pool = ctx.enter_context(tc.tile_pool(name="bw", bufs=NT))
        cv_pool = ctx.enter_context(tc.tile_pool(name="cv", bufs=CV_BUFS))
        x_pool = ctx.enter_context(tc.tile_pool(name="x", bufs=xbufs))
        y_pool = ctx.enter_context(tc.tile_pool(name="y", bufs=1))
        ps_pool = ctx.enter_context(
            tc.tile_pool(name="ps", bufs=8, space=bass.MemorySpace.PSUM)
        )
        st_pool = ctx.enter_context(tc.tile_pool(name="st", bufs=1))
        sg_pool = ctx.enter_context(tc.tile_pool(name="sg", bufs=4))
        sm_pool = ctx.enter_context(tc.tile_pool(name="sm", bufs=1))

        # PE warm-up: the HAM clock gate holds the PE at 1.2 GHz until it has
        # been busy ~3.4us. Burn dummy matmuls during the initial DMA wait so
        # the real matmul stream starts at 2.4 GHz.
        wl = sm_pool.tile([128, 64], bf16)
        wr = sm_pool.tile([128, 64], bf16)
        nc.vector.memset(wl[:], 0.0)
        nc.vector.memset(wr[:], 0.0)
        # 24 x ~110ns cold covers the DMA/conversion latency of the first
        # real tiles without queueing so much warmup that it delays them
        wp = ps_pool.tile([128, CH], f32, name="wups", tag="ps")
        for _ in range(24):
            nc.tensor.matmul(wp[0:64, 0:64], wl[:], wr[:], start=True, stop=True)

        bw_tiles = [None] * NT
        # All weight tiles ride the ACT HWDGE ring: ACT is idle at startup,
        # while Sync and GpSimd carry the x stream from t=0 (queuing bw
        # behind x there starved the first chunk-pair's conversions).
        for t in range(NT):
            bt = bw_pool.tile([128, OSH], bf16, name=f"bt{t}", tag="bt")
            nc.scalar.dma_start(bt[:], bwt_d.ap()[t])
            bw_tiles[t] = bt

        ct = sm_pool.tile([128, 4 * NOT], f32)
        nc.gpsimd.dma_start(ct[:], coef_d.ap())

        yt = [y_pool.tile([128, B], f32, name=f"yt{i}") for i in range(NOT)]
        stats = [st_pool.tile([128, 6 * NCH], f32, name=f"stats{i}") for i in range(NOT)]

        # Per-o-tile BN coefficients: with s-stats (mean_s, var_s) and host
        # precomputed p1=alpha^2, p2=alpha*gamma, p4=beta:
        #   inv = 1/sqrt(p1*var_s + eps);  A = p2*inv;  B = p4 - mean_s*A
        A_t, B_t, mv_t = [None] * NOT, [None] * NOT, [None] * NOT
        eps_t = sm_pool.tile([128, 1], f32)
        nc.vector.memset(eps_t[:], BN_EPS)

        def coef_chain(ot):
            mv = sm_pool.tile([128, 2], f32, name=f"mv{ot}", tag=f"mv{ot}")
            nc.vector.bn_aggr(mv[:], stats[ot][:])
            p1 = ct[:, ot : ot + 1]
            p2 = ct[:, NOT + ot : NOT + ot + 1]
            p4 = ct[:, 3 * NOT + ot : 3 * NOT + ot + 1]
            v = sm_pool.tile([128, 1], f32, name=f"v{ot}", tag=f"v{ot}")
            nc.vector.tensor_mul(v[:], mv[:, 1:2], p1)
            sd = sm_pool.tile([128, 1], f32, name=f"sd{ot}", tag=f"sd{ot}")
            nc.scalar.activation(
                sd[:], v[:], mybir.ActivationFunctionType.Sqrt, bias=eps_t[:]
            )
            inv = sm_pool.tile([128, 1], f32, name=f"inv{ot}", tag=f"inv{ot}")
            nc.vector.reciprocal(inv[:], sd[:])
            Ac = sm_pool.tile([128, 1], f32, name=f"Ac{ot}", tag=f"Ac{ot}")
            nc.vector.tensor_mul(Ac[:], p2, inv[:])
            mB = sm_pool.tile([128, 1], f32, name=f"mB{ot}", tag=f"mB{ot}")
            nc.vector.tensor_mul(mB[:], mv[:, 0:1], Ac[:])
            Bc = sm_pool.tile([128, 1], f32, name=f"Bc{ot}", tag=f"Bc{ot}")
            nc.vector.tensor_sub(Bc[:], p4, mB[:])
            A_t[ot], B_t[ot], mv_t[ot] = Ac, Bc, mv

        # Streaming weight conversion: cv[t] is a f32 copy of bw tile t,
        # refreshed per cpair; matmul reads it through a f32r bitcast.
        cvs = {}

        def conv(cp, t):
            # DVE upconvert bf16 -> f32r (the f32r output dtype makes DVE
            # round to FP22, which the BIR verifier requires for f32r
            # matmul inputs; +-1 weights are exact either way)
            cv = cv_pool.tile([128, OSH], f32r, name=f"cv{cp}_{t}", tag="cv")
            nc.vector.tensor_copy(cv[:], bw_tiles[t][:])
            cvs[t] = cv

        def drain_copies(cp):
            """Evacuate cpair cp's 8 PSUM banks (the copies are the only
            bank readers). Free banks in POOL-ALLOCATION order (ot-major)
            split across ACT/DVE so the next cpair's matmuls reacquire them
            with minimal stall (~0.6us per engine step, two engines)."""
            ps = ps_tiles[cp]
            for ot in range(NOT):
                for j in range(2):
                    c = 2 * cp + j
                    dst = yt[ot][:, c * CH : (c + 1) * CH]
                    if (2 * ot + j) % 2 == 0:
                        nc.scalar.copy(dst, ps[ot][j][:])
                    else:
                        nc.vector.tensor_copy(dst, ps[ot][j][:])

        def drain_stats(cp):
            """BN stats off the bank-free critical path: read the y copies.
            Emitted AFTER the next cpair's conversions so a waiting stats op
            never head-of-line blocks the DVE queue's CAST stream."""
            for ot in range(NOT):
                for j in range(2):
                    c = 2 * cp + j
                    nc.vector.bn_stats(
                        stats[ot][:, c * 6 : (c + 1) * 6],
                        yt[ot][:, c * CH : (c + 1) * CH],
                    )
                    if c == NCH - 1:
                        coef_chain(ot)

        ps_tiles = {}
        for cp in range(NCP):
            if cp == 1:
                # Preload the tail ACT LUTs (Sqrt, Sign) once the stream is
                # rolling: no ACT_TABLE_LOAD on the critical tail.
                wt = sm_pool.tile([128, 1], f32)
                nc.vector.memset(wt[:], 1.0)
                wt2 = sm_pool.tile([128, 1], f32)
                nc.scalar.activation(
                    wt2[:], wt[:], mybir.ActivationFunctionType.Sqrt
                )
                nc.scalar.activation(
                    wt2[:], wt[:], mybir.ActivationFunctionType.Sign
                )
            # convert the first tiles of THIS cpair before emitting the
            # previous cpair's drain, so DVE has them ready when the PE
            # restarts
            for t in range(min(CV_BUFS - 2, NT)):
                conv(cp, t)
            if cp > 0:
                drain_copies(cp - 1)
            # 8 live accumulators: (o-tile, chunk-of-pair) -> all 8 PSUM banks
            ps = [
                [
                    ps_pool.tile([128, CH], f32, name=f"ps{cp}_{i}_{j}", tag="ps")
                    for j in range(2)
                ]
                for i in range(NOT)
            ]
            ps_tiles[cp] = ps
            # x DMAs issued 2 tiles ahead of consumption so each tile's data
            # (and the pulled-ahead LDWEIGHTS) is resident before its first
            # matmul
            xts = {}

            def issue_x(t, cp=cp, xts=xts):
                xt = x_pool.tile([128, 2 * CH], f32r)
                # SP HWDGE + GpSimd SWDGE rings carry x; ACT is reserved for
                # the evacuation copies
                dma_eng = nc.sync if t % 2 == 0 else nc.gpsimd
                dma_eng.dma_start(xt[:], xtp_d.ap()[t, cp])
                xts[t] = xt

            # depth-4 prefetch: each 512KB x DMA gets ~7.3us of issue-to-need
            # slack (shallower depths left the PE exposed to DMA latency
            # jitter from 8 cores sharing the HBM stacks)
            for tp in range(4):
                issue_x(tp)
            for t in range(NT):
                if t + 4 < NT:
                    issue_x(t + 4)
                if t + CV_BUFS - 2 < NT:
                    conv(cp, t + CV_BUFS - 2)
                xt = xts.pop(t)
                cv = cvs.pop(t)
                for ot in range(NOT):
                    lhsT = cv[:, ot * 128 : (ot + 1) * 128]
                    for j in range(2):
                        nc.tensor.matmul(
                            ps[ot][j][:], lhsT, xt[:, j * CH : (j + 1) * CH],
                            start=(t == 0), stop=(t == NT - 1),
                        )
            if cp > 0:
                drain_stats(cp - 1)
        drain_copies(NCP - 1)
        drain_stats(NCP - 1)

        # Final affine+sign, split between ScalarE (one ACTIVATE(Sign) per
        # slab) and DVE so the serial tail drains on two engines at once.
        for k in range(NOT * NSG):
            h, ot = divmod(k, NOT)
            lo = h * SGW
            ysl = yt[ot][:, lo : lo + SGW]
            sg = sg_pool.tile([128, SGW], bf16, name=f"sg{k}", tag="sg")
            # DVE slab (~2.0us) vs ACT slab (~2.1us): 6 of 16 on DVE evens
            # the two engines' tail drain time.
            if k % 8 in (1, 4, 6) and simple_tail:
                # sg = (s >= mean) ; sg = 2*sg - 1
                nc.vector.tensor_scalar(
                    sg[:], ysl, mv_t[ot][:, 0:1], None,
                    mybir.AluOpType.is_ge,
                )
                nc.vector.tensor_scalar(
                    sg[:], sg[:], 2.0, 1.0,
                    mybir.AluOpType.mult, mybir.AluOpType.subtract,
                )
            else:
                nc.scalar.activation(
                    sg[:], ysl,
                    mybir.ActivationFunctionType.Sign,
                    bias=B_t[ot][:],
                    scale=A_t[ot][:],
                )
            # alternate output rings so out-DMAs don't serialize behind one
            # HWDGE queue (frees sg bufs sooner for the shallow pool)
            out_eng = nc.sync if k % 2 == 0 else nc.gpsimd
            out_eng.dma_start(
                out_d.ap()[ot * 128 : (ot + 1) * 128, lo : lo + SGW], sg[:]
            )

    nc.compile()
    return nc


def prep_inputs(x, w, gamma, beta, n_cores=N_CORES, CH=512):
    """Host-side prep: transpose/pair-pack x (f32), bw/coef shards per core."""
    B, I = x.shape
    O = w.shape[0]
    OSH = O // n_cores
    NT = I // 128
    NCH = B // CH

    xt = np.ascontiguousarray(x.T)                  # [I, B] f32
    # chunk-PAIR tiles: [NT, NCH//2, 128, 2CH] (b-contiguous 1024-wide)
    xtp = np.ascontiguousarray(
        xt.reshape(NT, 128, NCH // 2, 2 * CH).transpose(0, 2, 1, 3)
    )

    bw = np.sign(w).astype(np.float32)
    alpha = np.abs(w).mean(axis=1)                 # [O] f32
    p1 = alpha * alpha
    p2 = alpha * gamma
    p3 = alpha * alpha * gamma
    p4 = beta.astype(np.float32)

    in_maps = []
    for k in range(n_cores):
        sl = slice(k * OSH, (k + 1) * OSH)
        bwt = np.ascontiguousarray(bw[sl].T).reshape(NT, 128, OSH).astype(BF16)
        NOT = OSH // 128

        def per_tile(vec):
            return np.ascontiguousarray(vec[sl].reshape(NOT, 128).T)  # [128, NOT]

        coef = np.concatenate(
            [per_tile(p1), per_tile(p2), per_tile(p3), per_tile(p4)], axis=1
        ).astype(np.float32)
        in_maps.append({"xtp": xtp, "bwt": bwt, "coef": coef})
    return in_maps


_NC_CACHE = {}


def kernel(x, real_weight, gamma, beta):
    global LAST_RESULTS
    x = np.asarray(x, dtype=np.float32)
    w = np.asarray(real_weight, dtype=np.float32)
    gamma = np.asarray(gamma, dtype=np.float32)
    beta = np.asarray(beta, dtype=np.float32)
    B, I = x.shape
    O = w.shape[0]
    OSH = O // N_CORES
    CH = 512

    simple_tail = bool((gamma > 0).all() and (beta == 0).all())
    key = (B, I, OSH, CH, simple_tail)
    if key not in _NC_CACHE:
        _NC_CACHE[key] = build_nc(B, I, OSH, CH, simple_tail=simple_tail)
    nc = _NC_CACHE[key]

    in_maps = prep_inputs(x, w, gamma, beta, N_CORES, CH)
    trace = bool(int(os.environ.get("KERNEL_TRACE", "0")))
    res = run_bass_kernel_spmd(
        nc, in_maps, core_ids=list(range(N_CORES)), trace=trace
    )
    LAST_RESULTS = res

    out = np.empty((B, O), dtype=np.float32)
    for k in range(N_CORES):
        o = res.results[k]["out"]                  # [OSH, B] bf16
        out[:, k * OSH : (k + 1) * OSH] = o.T.astype(np.float32)
    return out
